# revision 5
# baseline (speedup 1.0000x reference)
"""Trainium2 Bass kernel for nn_IntraInterAtt (gnn_message_passing).

Strategy (8 NeuronCores, SPMD):
- The dominant op x5 @ att5a_w.T ([2048,96961]@[96961,1024]) is split along the
  contraction dim K across the 8 cores (bf16 operands, fp32 PSUM accum), each
  core folds its partial through att5b_w, then one 1MB AllReduce combines h5.
- h1..h4 are computed replicated; per-core LSTM + collapsed intra attention run
  data-parallel over 256 nodes (the intra 5-node attention is exactly uniform
  softmax, so it collapses to dense matmuls).
- Inter GCN / TransformerConv use a dense [src,dst] adjacency built on host from
  integer edge indices only (gather of edge weights into <=3 duplicate planes;
  counts matrix for softmax multiplicity via exp(S+ln C)); per-core dst slices,
  two small AllGathers exchange node features between stages.
"""
import numpy as np
import ml_dtypes
import concourse.bass as bass
import concourse.bacc as bacc
import concourse.tile as tile
import concourse.mybir as mybir
from concourse.bass_utils import run_bass_kernel_spmd

N, E, D, H, A, X5D = 2048, 32768, 128, 64, 5, 96961
NC = 8
NLOC = N // NC                 # 256 nodes per core
NKT = 96                       # k-tiles per core for the big matmul
KPC = NKT * 128                # 12288
KPAD = KPC * NC                # 98304
KGRP = 4                       # k-tiles per DMA group
NGRP = NKT // KGRP             # 24
NPASS = 4                      # node chunks of 512
NPLANES = 3
F32 = mybir.dt.float32
BF16 = mybir.dt.bfloat16
AF = mybir.ActivationFunctionType
AX = mybir.AxisListType
OP = mybir.AluOpType
BF = ml_dtypes.bfloat16
ISQ = 1.0 / float(np.sqrt(128.0))

_prog_cache = {}


def _build_program():
    if "nc" in _prog_cache:
        return _prog_cache["nc"]
    nc = bacc.Bacc("TRN2", target_bir_lowering=False, debug=False, num_devices=NC)
    RG = [list(range(NC))]

    def ext(name, shape, dt=F32):
        return nc.dram_tensor(name, shape, dt, kind="ExternalInput")

    x5t_d = ext("x5t", [KPC, N], BF16)
    w5at_d = ext("w5at", [KPC, 1024], BF16)
    w5bt_d = ext("w5bt", [1024, 128], BF16)
    b5af_d = ext("b5af", [128, 8])
    b5ah_d = ext("b5ah", [128, 8], BF16)
    b5b_d = ext("b5b", [128, 1])
    xs_d = [ext(f"x{i+1}t", [17 if i < 2 else 6, N]) for i in range(4)]
    aw_d = [ext(f"aw{i+1}t", [17 if i < 2 else 6, 128]) for i in range(4)]
    ab_d = [ext(f"ab{i+1}", [128, 1]) for i in range(4)]
    wihf_d = ext("wihf", [128, 256]); whhf_d = ext("whhf", [64, 256])
    wihb_d = ext("wihb", [128, 256]); whhb_d = ext("whhb", [64, 256])
    bif_d = ext("bif", [64, 4]); bhf_d = ext("bhf", [64, 4])
    bib_d = ext("bib", [64, 4]); bhb_d = ext("bhb", [64, 4])
    gcn1wt_d = ext("gcn1wt", [128, 128]); bgcn1_d = ext("bgcn1", [128, 1])
    itvwt_d = ext("itvwt", [128, 128]); itswt_d = ext("itswt", [128, 128])
    bitvs_d = ext("bitvs2", [128, 2])
    gcn2wt_d = ext("gcn2wt", [128, 128]); bgcn2bc_d = ext("bgcn2bc", [128, 128])
    etqwt_d = ext("etqwt", [128, 128]); betq_d = ext("betq", [128, 1])
    etkwt_d = ext("etkwt", [128, 128]); betk_d = ext("betk", [128, 1])
    etvwt_d = ext("etvwt", [128, 128]); betvbc_d = ext("betvbc", [128, 128])
    etswt_d = ext("etswt", [128, 128]); betsbc_d = ext("betsbc", [128, 128])
    fcwbc_d = ext("fcwbc", [128, 128]); fcb_d = ext("fcb", [128, 1])
    fcmwbc_d = ext("fcmwbc", [128, 128]); fcmb_d = ext("fcmb", [128, 1])
    eye_d = ext("eye", [128, 128])
    planes_d = ext("planes", [NPLANES, N, NLOC])
    cmat_d = ext("cmat", [NLOC, N])

    out1_d = nc.dram_tensor("out1", [128, 2], F32, kind="ExternalOutput")
    out2_d = nc.dram_tensor("out2", [128, 2], F32, kind="ExternalOutput")

    h5part_d = nc.dram_tensor("h5part", [128, N], F32)
    h5full_d = nc.dram_tensor("h5full", [128, N], F32, addr_space="Shared")
    h_all_d = nc.dram_tensor("h_all", [128, 5 * N], F32)
    degin_d = nc.dram_tensor("degin", [1, NLOC], F32)
    degout_d = nc.dram_tensor("degout", [NC, NLOC], F32, addr_space="Shared")
    ia_in_d = nc.dram_tensor("ia_in", [128, NLOC], F32)
    ia_out_d = nc.dram_tensor("ia_out", [NC * 128, NLOC], F32, addr_space="Shared")
    it_in_d = nc.dram_tensor("it_in", [128, NLOC], F32)
    it_out_d = nc.dram_tensor("it_out", [NC * 128, NLOC], F32, addr_space="Shared")

    with tile.TileContext(nc) as tc:
        with (
            tc.tile_pool(name="const", bufs=1) as cp,
            tc.tile_pool(name="persist", bufs=1) as pp,
            tc.tile_pool(name="xg", bufs=2) as pgx,
            tc.tile_pool(name="wg", bufs=2) as pgw,
            tc.tile_pool(name="psA", bufs=8, space="PSUM") as psA,
            tc.tile_pool(name="work", bufs=2) as wk,
            tc.tile_pool(name="big", bufs=1) as bigp,
        ):
            pid = nc.partition_id()

            # ---- constants in SBUF ----
            def cload(dram, shape, dt=F32):
                t = cp.tile(shape, dt, tag=dram.name)
                nc.sync.dma_start(t[:], dram[:])
                return t

            w5bt_sb = cp.tile([128, 8, 128], BF16, tag="w5bt")
            nc.sync.dma_start(w5bt_sb[:], w5bt_d[:].rearrange("(m p) d -> p m d", p=128))
            b5af = cload(b5af_d, [128, 8]); b5ah = cload(b5ah_d, [128, 8], BF16)
            b5b = cload(b5b_d, [128, 1])
            aw_sb = [cload(aw_d[i], [17 if i < 2 else 6, 128]) for i in range(4)]
            ab_sb = [cload(ab_d[i], [128, 1]) for i in range(4)]
            wih_sb = [cload(wihf_d, [128, 256]), cload(wihb_d, [128, 256])]
            whh_sb = [cload(whhf_d, [64, 256]), cload(whhb_d, [64, 256])]
            bif = cload(bif_d, [64, 4]); bhf = cload(bhf_d, [64, 4])
            bib = cload(bib_d, [64, 4]); bhb = cload(bhb_d, [64, 4])
            gcn1wt = cload(gcn1wt_d, [128, 128]); bgcn1 = cload(bgcn1_d, [128, 1])
            itvwt = cload(itvwt_d, [128, 128]); itswt = cload(itswt_d, [128, 128])
            bitvs = cload(bitvs_d, [128, 2])
            gcn2wt = cload(gcn2wt_d, [128, 128]); bgcn2bc = cload(bgcn2bc_d, [128, 128])
            etqwt = cload(etqwt_d, [128, 128]); betq = cload(betq_d, [128, 1])
            etkwt = cload(etkwt_d, [128, 128]); betk = cload(betk_d, [128, 1])
            etvwt = cload(etvwt_d, [128, 128]); betvbc = cload(betvbc_d, [128, 128])
            etswt = cload(etswt_d, [128, 128]); betsbc = cload(betsbc_d, [128, 128])
            fcwbc = cload(fcwbc_d, [128, 128]); fcb = cload(fcb_d, [128, 1])
            fcmwbc = cload(fcmwbc_d, [128, 128]); fcmb = cload(fcmb_d, [128, 1])
            eye = cload(eye_d, [128, 128])
            one1 = cp.tile([1, 1], F32, tag="one1")
            nc.gpsimd.memset(one1[:], 1.0)
            ones128 = cp.tile([128, 1], F32, tag="ones128")
            nc.gpsimd.memset(ones128[:], 1.0)
            # derived constants
            gcn1wt_s = cp.tile([128, 128], F32, tag="g1s")
            nc.vector.tensor_scalar_mul(gcn1wt_s[:], gcn1wt[:], 0.2)
            wvs = cp.tile([128, 128], F32, tag="wvs")
            nc.vector.tensor_add(wvs[:], itvwt[:], itswt[:])
            bsum = [cp.tile([64, 4], F32, tag=f"bsum{d}", name=f"bsum{d}") for d in range(2)]
            nc.vector.tensor_add(bsum[0][:], bif[:], bhf[:])
            nc.vector.tensor_add(bsum[1][:], bib[:], bhb[:])

            # ---- Phase A: big matmul, K-sharded ----
            h5part = pp.tile([128, N], F32, tag="h5part")
            for pas in range(NPASS):
                accs = [psA.tile([128, 512], F32, tag="ps", name=f"acc{pas}_{m}") for m in range(8)]
                for g in range(NGRP):
                    xg = pgx.tile([128, KGRP, 512], BF16, tag="xg")
                    nc.sync.dma_start(
                        xg[:],
                        x5t_d[g * 512:(g + 1) * 512, pas * 512:(pas + 1) * 512]
                        .rearrange("(j p) f -> p j f", p=128))
                    wg = pgw.tile([128, KGRP, 1024], BF16, tag="wg")
                    nc.sync.dma_start(
                        wg[:],
                        w5at_d[g * 512:(g + 1) * 512, :]
                        .rearrange("(j p) f -> p j f", p=128))
                    for j in range(KGRP):
                        for m in range(8):
                            nc.tensor.matmul(
                                accs[m][:], wg[:, j, m * 128:(m + 1) * 128], xg[:, j, :],
                                start=(g == 0 and j == 0),
                                stop=(g == NGRP - 1 and j == KGRP - 1))
                h5a_sb = []
                for m in range(8):
                    t = wk.tile([128, 512], BF16, tag=f"h5a{m}", bufs=1, name=f"h5a_{m}")
                    nc.vector.tensor_scalar_add(t[:], accs[m][:], b5af[:, m:m + 1])
                    h5a_sb.append(t)
                ph5 = psA.tile([128, 512], F32, tag="ps")
                for m in range(8):
                    nc.tensor.matmul(ph5[:], w5bt_sb[:, m, :], h5a_sb[m][:],
                                     start=(m == 0), stop=(m == 7))
                nc.vector.tensor_copy(h5part[:, pas * 512:(pas + 1) * 512], ph5[:])
            nc.sync.dma_start(h5part_d[:], h5part[:])
            nc.gpsimd.collective_compute("AllReduce", OP.add, replica_groups=RG,
                                         ins=[h5part_d[:]], outs=[h5full_d[:]])

            # ---- Phase B: h1..h4 replicated -> h_all ----
            for i in range(4):
                ki = 17 if i < 2 else 6
                xi = wk.tile([ki, N], F32, tag="xi")
                nc.sync.dma_start(xi[:], xs_d[i][:])
                for ch in range(4):
                    ps = psA.tile([128, 512], F32, tag="ps", name=f"psB{i}_{ch}")
                    nc.tensor.matmul(ps[:], aw_sb[i][:], xi[:, ch * 512:(ch + 1) * 512],
                                     start=True, stop=True)
                    hsb = wk.tile([128, 512], F32, tag="hsb")
                    nc.vector.tensor_scalar_add(hsb[:], ps[:], ab_sb[i][:])
                    nc.sync.dma_start(h_all_d[:, i * N + ch * 512: i * N + (ch + 1) * 512], hsb[:])

            # ---- adjacency planes, degree, lnC (independent of h5) ----
            WT = bigp.tile([128, 16, NLOC], F32, tag="WT")
            nc.sync.dma_start(WT[:], planes_d[0].rearrange("(j p) c -> p j c", p=128))
            for pl in range(1, NPLANES):
                for qk in range(4):
                    tmp = wk.tile([128, 4, NLOC], F32, tag="pltmp", name=f"pl{pl}_{qk}")
                    nc.sync.dma_start(
                        tmp[:],
                        planes_d[pl, qk * 512:(qk + 1) * 512, :]
                        .rearrange("(j p) c -> p j c", p=128))
                    nc.vector.tensor_add(WT[:, qk * 4:(qk + 1) * 4, :],
                                         WT[:, qk * 4:(qk + 1) * 4, :], tmp[:])
            psdeg = psA.tile([1, NLOC], F32, tag="ps")
            for j in range(16):
                nc.tensor.matmul(psdeg[:], ones128[:], WT[:, j, :],
                                 start=(j == 0), stop=(j == 15))
            degrow = pp.tile([1, NLOC], F32, tag="degrow")
            nc.vector.tensor_scalar_add(degrow[:], psdeg[:], 1.0)
            nc.sync.dma_start(degin_d[:], degrow[:])
            nc.gpsimd.collective_compute("AllGather", OP.bypass, replica_groups=RG,
                                         ins=[degin_d[:]], outs=[degout_d[:]])
            degown = wk.tile([128, 2], F32, tag="degown")
            for ct in range(2):
                pst = psA.tile([128, 1], F32, tag="ps", name=f"pst{ct}")
                nc.tensor.matmul(pst[:], degrow[0:1, ct * 128:(ct + 1) * 128], one1[:],
                                 is_transpose=True)
                nc.vector.tensor_copy(degown[:, ct:ct + 1], pst[:])
            sqown = wk.tile([128, 2], F32, tag="sqown")
            nc.scalar.activation(sqown[:], degown[:], AF.Sqrt)
            dinvown = pp.tile([128, 2], F32, tag="dinvown")
            nc.vector.reciprocal(dinvown[:], sqown[:])
            dinv2own = pp.tile([128, 2], F32, tag="dinv2own")
            nc.vector.tensor_mul(dinv2own[:], dinvown[:], dinvown[:])
            deg_sb = wk.tile([128, 16], F32, tag="deg_sb")
            nc.sync.dma_start(
                deg_sb[:],
                degout_d[:].rearrange("a c -> (a c)").rearrange("(j p) -> p j", p=128))
            sq16 = wk.tile([128, 16], F32, tag="sq16")
            nc.scalar.activation(sq16[:], deg_sb[:], AF.Sqrt)
            dinv_sb = pp.tile([128, 16], F32, tag="dinv_sb")
            nc.vector.reciprocal(dinv_sb[:], sq16[:])


            # ---- h5 bias + h_all assembly ----
            psb5 = psA.tile([128, 1], F32, tag="ps")
            for m in range(8):
                nc.tensor.matmul(psb5[:], w5bt_sb[:, m, :], b5ah[:, m:m + 1],
                                 start=(m == 0), stop=(m == 7))
            bias5 = wk.tile([128, 1], F32, tag="bias5")
            nc.vector.tensor_scalar_add(bias5[:], psb5[:], b5b[:])
            hf_sb = bigp.tile([128, N], F32, tag="winbuf", name="hf_sb")
            nc.sync.dma_start(hf_sb[:], h5full_d[:])
            nc.vector.tensor_scalar_add(hf_sb[:], hf_sb[:], bias5[:])
            nc.sync.dma_start(h_all_d[:, 4 * N:5 * N], hf_sb[:])

            # ---- window + LSTM ----
            off = pid * (5 * NLOC)
            win = bigp.tile([128, 5 * NLOC], F32, tag="winbuf", name="win")
            nc.sync.dma_start(win[:], h_all_d[:, bass.ds(off, 5 * NLOC)])
            xt = []
            for t in range(5):
                xtt = pp.tile([128, NLOC], F32, tag=f"xt{t}")
                nc.vector.tensor_copy(
                    xtt[:], win[:].rearrange("p (n t) -> p t n", t=5)[:, t, :])
                xt.append(xtt)
            sum_hd = [None, None]
            for d in range(2):
                hprev = None
                cprev = None
                ssum_t = pp.tile([64, NLOC], F32, tag=f"sumh{d}", name=f"sumh{d}")
                nc.gpsimd.memset(ssum_t[:], 0.0)
                sum_hd[d] = ssum_t
                for s in range(5):
                    tok = s if d == 0 else 4 - s
                    gates = []
                    for gi in range(4):
                        pg = psA.tile([64, NLOC], F32, tag="ps", name=f"g{d}{s}{gi}")
                        nc.tensor.matmul(pg[:], wih_sb[d][:, gi * 64:(gi + 1) * 64],
                                         xt[tok][:], start=True, stop=(s == 0))
                        if s > 0:
                            nc.tensor.matmul(pg[:], whh_sb[d][:, gi * 64:(gi + 1) * 64],
                                             hprev[:], start=False, stop=True)
                        gates.append(pg)
                    sig_i = wk.tile([64, NLOC], F32, tag="sig_i", bufs=1)
                    sig_f = wk.tile([64, NLOC], F32, tag="sig_f", bufs=1)
                    tanh_g = wk.tile([64, NLOC], F32, tag="tanh_g", bufs=1)
                    sig_o = wk.tile([64, NLOC], F32, tag="sig_o", bufs=1)
                    nc.scalar.activation(sig_i[:], gates[0][:], AF.Sigmoid, bias=bsum[d][:, 0:1])
                    nc.scalar.activation(sig_f[:], gates[1][:], AF.Sigmoid, bias=bsum[d][:, 1:2])
                    nc.scalar.activation(tanh_g[:], gates[2][:], AF.Tanh, bias=bsum[d][:, 2:3])
                    nc.scalar.activation(sig_o[:], gates[3][:], AF.Sigmoid, bias=bsum[d][:, 3:4])
                    t1 = wk.tile([64, NLOC], F32, tag="t1", bufs=1)
                    nc.vector.tensor_mul(t1[:], sig_i[:], tanh_g[:])
                    cnew = wk.tile([64, NLOC], F32, tag=f"c{s % 2}", bufs=1, name=f"c{d}{s}")
                    if s == 0:
                        nc.vector.tensor_copy(cnew[:], t1[:])
                    else:
                        t2 = wk.tile([64, NLOC], F32, tag="t2", bufs=1)
                        nc.vector.tensor_mul(t2[:], sig_f[:], cprev[:])
                        nc.vector.tensor_add(cnew[:], t1[:], t2[:])
                    thc = wk.tile([64, NLOC], F32, tag="thc", bufs=1)
                    nc.scalar.activation(thc[:], cnew[:], AF.Tanh)
                    hnew = wk.tile([64, NLOC], F32, tag=f"h{s % 2}", bufs=1, name=f"h{d}{s}")
                    nc.vector.tensor_mul(hnew[:], sig_o[:], thc[:])
                    nc.vector.tensor_add(ssum_t[:], ssum_t[:], hnew[:])
                    hprev, cprev = hnew, cnew
            sum_h = pp.tile([128, NLOC], F32, tag="sum_h")
            nc.sync.dma_start(sum_h[0:64, :], sum_hd[0][:])
            nc.sync.dma_start(sum_h[64:128, :], sum_hd[1][:])

            # ---- intra collapse ----
            psE = psA.tile([128, NLOC], F32, tag="ps")
            nc.tensor.matmul(psE[:], gcn1wt_s[:], sum_h[:], start=True, stop=True)
            g_sb = wk.tile([128, NLOC], F32, tag="g_sb")
            nc.vector.tensor_scalar_add(g_sb[:], psE[:], bgcn1[:])
            psE2 = psA.tile([128, NLOC], F32, tag="ps")
            nc.tensor.matmul(psE2[:], wvs[:], g_sb[:], start=True, stop=True)
            intra_sb = pp.tile([128, NLOC], F32, tag="intra_sb")
            bvs = wk.tile([128, 1], F32, tag="bvs")
            nc.vector.tensor_add(bvs[:], bitvs[:, 0:1], bitvs[:, 1:2])
            nc.vector.tensor_scalar_add(intra_sb[:], psE2[:], bvs[:])
            nc.sync.dma_start(ia_in_d[:], intra_sb[:])
            nc.gpsimd.collective_compute("AllGather", OP.bypass, replica_groups=RG,
                                         ins=[ia_in_d[:]], outs=[ia_out_d[:]])
            iab = bigp.tile([128, 8, NLOC], F32, tag="nodeblocks", name="iab")
            nc.sync.dma_start(iab[:], ia_out_d[:].rearrange("(r p) n -> p r n", p=128))

            # ---- GCN ----
            y_sb = bigp.tile([128, 16, 128], F32, tag="y_sb")
            for j in range(16):
                psf = psA.tile([128, 128], F32, tag="ps", name=f"psf{j}")
                nc.tensor.matmul(psf[:], iab[:, j // 2, (j % 2) * 128:(j % 2) * 128 + 128],
                                 gcn2wt[:], start=True, stop=True)
                nc.vector.tensor_scalar_mul(y_sb[:, j, :], psf[:], dinv_sb[:, j:j + 1])
            xwo = wk.tile([128, 2, 128], F32, tag="xwo")
            for ct in range(2):
                psf = psA.tile([128, 128], F32, tag="ps", name=f"psfo{ct}")
                nc.tensor.matmul(psf[:], intra_sb[:, ct * 128:(ct + 1) * 128], gcn2wt[:],
                                 start=True, stop=True)
                nc.vector.tensor_copy(xwo[:, ct, :], psf[:])
            inter0 = pp.tile([128, 2, 128], F32, tag="inter0")
            inter0T = pp.tile([128, NLOC], F32, tag="inter0T")
            for ct in range(2):
                psag = psA.tile([128, 128], F32, tag="ps", name=f"psag{ct}")
                for j in range(16):
                    nc.tensor.matmul(psag[:], WT[:, j, ct * 128:(ct + 1) * 128],
                                     y_sb[:, j, :], start=(j == 0), stop=(j == 15))
                s1 = wk.tile([128, 128], F32, tag="s1")
                nc.vector.scalar_tensor_tensor(s1[:], psag[:], dinvown[:, ct:ct + 1],
                                               bgcn2bc[:], op0=OP.mult, op1=OP.add)
                s2 = wk.tile([128, 128], F32, tag="s2")
                nc.vector.tensor_scalar_mul(s2[:], xwo[:, ct, :], dinv2own[:, ct:ct + 1])
                nc.vector.tensor_add(inter0[:, ct, :], s1[:], s2[:])
                pstr = psA.tile([128, 128], F32, tag="ps", name=f"pstri{ct}")
                nc.tensor.matmul(pstr[:], inter0[:, ct, :], eye[:], is_transpose=True)
                nc.vector.tensor_copy(inter0T[:, ct * 128:(ct + 1) * 128], pstr[:])
            nc.sync.dma_start(it_in_d[:], inter0T[:])
            nc.gpsimd.collective_compute("AllGather", OP.bypass, replica_groups=RG,
                                         ins=[it_in_d[:]], outs=[it_out_d[:]])
            i0b = bigp.tile([128, 8, NLOC], F32, tag="nodeblocks", name="i0b")
            nc.sync.dma_start(i0b[:], it_out_d[:].rearrange("(r p) n -> p r n", p=128))

            # ---- TransformerConv ----
            kT = bigp.tile([128, 8, NLOC], F32, tag="kT")
            for r in range(8):
                psg = psA.tile([128, NLOC], F32, tag="ps", name=f"psk{r}")
                nc.tensor.matmul(psg[:], etkwt[:], i0b[:, r, :], start=True, stop=True)
                nc.vector.tensor_scalar_add(kT[:, r, :], psg[:], betk[:])
            v_sb = bigp.tile([128, 16, 128], F32, tag="v_sb")
            for j in range(16):
                psg = psA.tile([128, 128], F32, tag="ps", name=f"psv{j}")
                nc.tensor.matmul(psg[:], i0b[:, j // 2, (j % 2) * 128:(j % 2) * 128 + 128],
                                 etvwt[:], start=True, stop=True)
                nc.vector.tensor_add(v_sb[:, j, :], psg[:], betvbc[:])
            qT = wk.tile([128, NLOC], F32, tag="qT")
            for ct in range(2):
                psg = psA.tile([128, 128], F32, tag="ps", name=f"psqq{ct}")
                nc.tensor.matmul(psg[:], etqwt[:], inter0T[:, ct * 128:(ct + 1) * 128],
                                 start=True, stop=True)
                nc.vector.tensor_scalar_add(qT[:, ct * 128:(ct + 1) * 128], psg[:], betq[:])
            o1_sb = wk.tile([128, 2], F32, tag="o1_sb")
            o2_sb = wk.tile([128, 2], F32, tag="o2_sb")
            for ct in range(2):
                lnc = bigp.tile([128, N], F32, tag="lnc", name=f"lnc{ct}")
                nc.sync.dma_start(
                    lnc[:],
                    cmat_d[ct * 128:(ct + 1) * 128, :])
                nc.scalar.activation(lnc[:], lnc[:], AF.Ln)
                Sm = lnc
                for nch in range(4):
                    psS = psA.tile([128, 512], F32, tag="ps", name=f"psS{ct}_{nch}")
                    nc.tensor.matmul(psS[:], qT[:, ct * 128:(ct + 1) * 128],
                                     kT[:, 2 * nch:2 * nch + 2, :], start=True, stop=True)
                    nc.vector.scalar_tensor_tensor(
                        Sm[:, nch * 512:(nch + 1) * 512], psS[:], ISQ,
                        lnc[:, nch * 512:(nch + 1) * 512], op0=OP.mult, op1=OP.add)
                negm = wk.tile([128, 1], F32, tag="negm")
                nc.vector.tensor_reduce(negm[:], Sm[:], axis=AX.X, op=OP.max, negate=True)
                Aun = bigp.tile([128, N], F32, tag="Aun", name=f"Aun{ct}")
                ssum = wk.tile([128, 1], F32, tag="ssum")
                nc.scalar.activation(Aun[:], Sm[:], AF.Exp, bias=negm[:], accum_out=ssum[:])
                rs = wk.tile([128, 1], F32, tag="rs")
                nc.vector.reciprocal(rs[:], ssum[:])
                AT = bigp.tile([128, 16, 128], F32, tag="AT", name=f"AT{ct}")
                for j in range(16):
                    pstr = psA.tile([128, 128], F32, tag="ps", name=f"ptA{ct}_{j}")
                    nc.tensor.matmul(pstr[:], Aun[:, j * 128:(j + 1) * 128], eye[:],
                                     is_transpose=True)
                    nc.vector.tensor_copy(AT[:, j, :], pstr[:])
                psat = psA.tile([128, 128], F32, tag="ps", name=f"psat{ct}")
                for j in range(16):
                    nc.tensor.matmul(psat[:], AT[:, j, :],
                                     v_sb[:, j, :], start=(j == 0), stop=(j == 15))
                atn = wk.tile([128, 128], F32, tag="atn")
                nc.vector.tensor_scalar_mul(atn[:], psat[:], rs[:])
                pssk = psA.tile([128, 128], F32, tag="ps", name=f"pssk{ct}")
                nc.tensor.matmul(pssk[:], inter0T[:, ct * 128:(ct + 1) * 128], etswt[:],
                                 start=True, stop=True)
                sk = wk.tile([128, 128], F32, tag="sk")
                nc.vector.tensor_add(sk[:], pssk[:], betsbc[:])
                outt = wk.tile([128, 128], F32, tag="outt")
                nc.vector.tensor_add(outt[:], atn[:], sk[:])
                m1 = wk.tile([128, 128], F32, tag="m1")
                nc.vector.tensor_mul(m1[:], outt[:], fcwbc[:])
                r1 = wk.tile([128, 1], F32, tag="r1")
                nc.vector.tensor_reduce(r1[:], m1[:], axis=AX.X, op=OP.add)
                nc.vector.tensor_scalar_add(o1_sb[:, ct:ct + 1], r1[:], fcb[:])
                m2 = wk.tile([128, 128], F32, tag="m2")
                nc.vector.tensor_mul(m2[:], outt[:], fcmwbc[:])
                r2 = wk.tile([128, 1], F32, tag="r2")
                nc.vector.tensor_reduce(r2[:], m2[:], axis=AX.X, op=OP.add)
                nc.vector.tensor_scalar_add(o2_sb[:, ct:ct + 1], r2[:], fcmb[:])
            nc.sync.dma_start(out1_d[:], o1_sb[:])
            nc.sync.dma_start(out2_d[:], o2_sb[:])

    nc.compile()
    _prog_cache["nc"] = nc
    return nc


def _prep_inputs(x1, x2, x3, x4, x5, edge_index, edge_wt_gaze, params):
    p = params
    f32 = np.float32

    def T(a):
        return np.ascontiguousarray(np.asarray(a, f32).T)

    x5t_full = np.zeros((KPAD, N), BF)
    x5t_full[:X5D] = np.asarray(x5, f32).T.astype(BF)
    w5at_full = np.zeros((KPAD, 1024), BF)
    w5at_full[:X5D] = np.asarray(p["att5a_w"], f32).T.astype(BF)

    def colmaj(v, nt):  # [nt*128] -> [128, nt] tile-major
        return np.ascontiguousarray(np.asarray(v, f32).reshape(nt, 128).T)

    common = {
        "w5bt": T(p["att5b_w"]).astype(BF),
        "b5af": colmaj(p["att5a_b"], 8),
        "b5ah": colmaj(p["att5a_b"], 8).astype(BF),
        "b5b": np.asarray(p["att5b_b"], f32).reshape(128, 1),
        "x1t": T(x1), "x2t": T(x2), "x3t": T(x3), "x4t": T(x4),
        "aw1t": T(p["att1_w"]), "aw2t": T(p["att2_w"]),
        "aw3t": T(p["att3_w"]), "aw4t": T(p["att4_w"]),
        "ab1": np.asarray(p["att1_b"], f32).reshape(128, 1),
        "ab2": np.asarray(p["att2_b"], f32).reshape(128, 1),
        "ab3": np.asarray(p["att3_b"], f32).reshape(128, 1),
        "ab4": np.asarray(p["att4_b"], f32).reshape(128, 1),
        "wihf": T(p["lstm_f_wih"]), "whhf": T(p["lstm_f_whh"]),
        "wihb": T(p["lstm_b_wih"]), "whhb": T(p["lstm_b_whh"]),
        "bif": np.ascontiguousarray(np.asarray(p["lstm_f_bih"], f32).reshape(4, 64).T),
        "bhf": np.ascontiguousarray(np.asarray(p["lstm_f_bhh"], f32).reshape(4, 64).T),
        "bib": np.ascontiguousarray(np.asarray(p["lstm_b_bih"], f32).reshape(4, 64).T),
        "bhb": np.ascontiguousarray(np.asarray(p["lstm_b_bhh"], f32).reshape(4, 64).T),
        "gcn1wt": T(p["gcn1_w"]),
        "bgcn1": np.asarray(p["gcn1_b"], f32).reshape(128, 1),
        "itvwt": T(p["it_v_w"]), "itswt": T(p["it_s_w"]),
        "bitvs2": np.stack([np.asarray(p["it_v_b"], f32),
                            np.asarray(p["it_s_b"], f32)], axis=1),
        "gcn2wt": T(p["gcn2_w"]),
        "bgcn2bc": np.ascontiguousarray(
            np.broadcast_to(np.asarray(p["gcn2_b"], f32), (128, 128))),
        "etqwt": T(p["et_q_w"]), "betq": np.asarray(p["et_q_b"], f32).reshape(128, 1),
        "etkwt": T(p["et_k_w"]), "betk": np.asarray(p["et_k_b"], f32).reshape(128, 1),
        "etvwt": T(p["et_v_w"]),
        "betvbc": np.ascontiguousarray(
            np.broadcast_to(np.asarray(p["et_v_b"], f32), (128, 128))),
        "etswt": T(p["et_s_w"]),
        "betsbc": np.ascontiguousarray(
            np.broadcast_to(np.asarray(p["et_s_b"], f32), (128, 128))),
        "fcwbc": np.ascontiguousarray(
            np.broadcast_to(np.asarray(p["fc_w"], f32).reshape(1, 128), (128, 128))),
        "fcb": np.full((128, 1), np.asarray(p["fc_b"], f32).reshape(())[()], f32),
        "fcmwbc": np.ascontiguousarray(
            np.broadcast_to(np.asarray(p["fcm_w"], f32).reshape(1, 128), (128, 128))),
        "fcmb": np.full((128, 1), np.asarray(p["fcm_b"], f32).reshape(())[()], f32),
        "eye": np.eye(128, dtype=f32),
    }

    # adjacency planes + counts (integer index manipulation only)
    src = np.asarray(edge_index[0], np.int64)
    dst = np.asarray(edge_index[1], np.int64)
    order = np.lexsort((src, dst))
    s_s, s_d = src[order], dst[order]
    same = np.zeros(E, bool)
    same[1:] = (s_s[1:] == s_s[:-1]) & (s_d[1:] == s_d[:-1])
    run_start = np.where(~same)[0]
    rank = np.arange(E) - np.repeat(run_start, np.diff(np.append(run_start, E)))
    assert int(rank.max()) + 1 <= NPLANES
    planes_idx = np.full((NPLANES, N, N), E, np.int32)
    planes_idx[rank, s_s, s_d] = order.astype(np.int32)
    w_ext = np.concatenate([np.asarray(edge_wt_gaze, f32), [0.0]]).astype(f32)
    planes_full = w_ext[planes_idx]              # [NPLANES, src, dst]
    Cfull = np.zeros((N, N), np.int32)           # [dst, src]
    np.add.at(Cfull, (dst, src), 1)
    Cfull = Cfull.astype(f32)

    in_maps = []
    for c in range(NC):
        m = dict(common)
        m["x5t"] = np.ascontiguousarray(x5t_full[c * KPC:(c + 1) * KPC])
        m["w5at"] = np.ascontiguousarray(w5at_full[c * KPC:(c + 1) * KPC])
        m["planes"] = np.ascontiguousarray(planes_full[:, :, c * NLOC:(c + 1) * NLOC])
        m["cmat"] = np.ascontiguousarray(Cfull[c * NLOC:(c + 1) * NLOC, :])
        in_maps.append(m)
    return in_maps


def kernel(x1, x2, x3, x4, x5, edge_index, edge_wt_gaze, edge_wt_sp, batch, params):
    nc = _build_program()
    in_maps = _prep_inputs(x1, x2, x3, x4, x5, edge_index, edge_wt_gaze, params)
    res = run_bass_kernel_spmd(nc, in_maps, list(range(NC)))
    o1 = np.concatenate(
        [res.results[c]["out1"].T.reshape(NLOC) for c in range(NC)])[:, None]
    o2 = np.concatenate(
        [res.results[c]["out2"].T.reshape(NLOC) for c in range(NC)])[:, None]
    return o1.astype(np.float32), o2.astype(np.float32)
